# revision 22
# baseline (speedup 1.0000x reference)
"""Trainium2 Bass kernel for nn_Attention (B=64, N=289, C=768, H=12).

Data-parallel over batch: 8 batches per NeuronCore x 8 cores, no collectives.

v2 pipeline (per core, all matmuls bf16 with f32 PSUM accumulation):
  1. qkv GEMM token-major; q/k weights are host-centered per head (so the
     LN mean vanishes) and scaled by 1/8 (so sum(q^2) over d equals the
     biased variance directly).
  2. LN stats: ScalarE Square (PSUM->SBUF), DVE reduce -> u = var,
     ScalarE Sqrt(u*scale + eps) with folded constants, DVE reciprocal.
  3. apply: DVE scalar_tensor_tensor y = ps * s'' (per-head broadcast),
     z2 = y + beta' (beta' = bias/gain pre-rotation, constant table),
     rot = z2*CC + swap(z2)*SS with gain-folded cos/sin tables; the two
     swap-half products run on GPSIMD.
  4. ONE xbar DMA transpose per 128-token tile ([128,1536] -> [128,12,128])
     lands q/k feature-major strips directly (block fold = contiguous
     strips, validated on HW).
  5. scores transposed ST[k_tok, q_tok] per head pair, two heads packed in
     the PE via row tile_position; exp on ScalarE from PSUM.
  6. AV with a ones-column appended to V: OT[65, q], row 64 = softmax sums;
     sums gathered (ScalarE/DVE split), one batched reciprocal per batch,
     partition-broadcast of reciprocal rows via a selector matmul, then
     normalize into bf16 OTn.
  7. proj GEMM feature-major + bias; one batched DMA out per batch.
Phase A tiles and phase B batches are interleaved at sub-batch granularity
(round-robin generators) so exp/DVE chains hide behind PE GEMM streams.
"""

import sys

if "/opt/trn_rl_repo" not in sys.path:
    sys.path.insert(0, "/opt/trn_rl_repo")

from contextlib import ExitStack

import ml_dtypes
import numpy as np

import concourse.bass as bass
import concourse.tile as tile
from concourse import bacc, mybir
from concourse.bass_utils import run_bass_kernel_spmd

F32 = mybir.dt.float32
I32 = mybir.dt.int32
BF16 = mybir.dt.bfloat16
BF = ml_dtypes.bfloat16
OP = mybir.AluOpType
AF = mybir.ActivationFunctionType

B, N, C, H = 64, 289, 768, 12
D = C // H  # 64
NCORES = 8
BPC = B // NCORES  # 8 batches per core
T = BPC * N  # 2312 tokens per core
NT = (T + 127) // 128  # 19 token blocks
TPAD = NT * 128  # 2432
EPS = 1e-5

_CACHE = {}


def _batch_blocks():
    out, r = [], 0
    while r < N:
        rows = min(128, N - r)
        out.append((r, rows))
        r += rows
    return out


def _build_program():
    nc = bacc.Bacc("TRN2", target_bir_lowering=False, debug=False,
                   num_devices=NCORES)

    xT = nc.dram_tensor("xT", [C, TPAD], BF16, kind="ExternalInput").ap()
    wqkvT = nc.dram_tensor("wqkvT", [C, 6 * 384], BF16,
                           kind="ExternalInput").ap()
    wprojT = nc.dram_tensor("wprojT", [C, C], BF16, kind="ExternalInput").ap()
    pbias = nc.dram_tensor("pbias", [C], F32, kind="ExternalInput").ap()
    # rope+gain tables, order: qCC, qSS, qB2, kCC, kSS, kB2  [6, TPAD, D]
    tabs = nc.dram_tensor("tabs", [6, TPAD, D], BF16,
                          kind="ExternalInput").ap()
    sel = nc.dram_tensor("sel", [12, C], BF16, kind="ExternalInput").ap()
    out = nc.dram_tensor("out", [C, T], F32, kind="ExternalOutput").ap()

    kblocks = _batch_blocks()

    with tile.TileContext(nc) as tc, ExitStack() as ctx:
        consts = ctx.enter_context(tc.tile_pool(name="consts", bufs=1))
        work = ctx.enter_context(tc.tile_pool(name="work", bufs=3))
        stat = ctx.enter_context(tc.tile_pool(name="stat", bufs=3))
        rotp = ctx.enter_context(tc.tile_pool(name="rotp", bufs=2))
        strips = ctx.enter_context(tc.tile_pool(name="strips", bufs=1))
        vpool = ctx.enter_context(tc.tile_pool(name="vpool", bufs=8))
        ptpool = ctx.enter_context(tc.tile_pool(name="ptpool", bufs=11))
        otnpool = ctx.enter_context(tc.tile_pool(name="otnpool", bufs=16))
        bpool = ctx.enter_context(tc.tile_pool(name="bpool", bufs=2))
        ysbp = ctx.enter_context(tc.tile_pool(name="ysbp", bufs=2))
        ypool = ctx.enter_context(tc.tile_pool(name="ypool", bufs=2))
        mmps = ctx.enter_context(tc.tile_pool(name="mmps", bufs=2,
                                              space="PSUM"))
        scps = ctx.enter_context(tc.tile_pool(name="scps", bufs=2,
                                              space="PSUM"))
        otps = ctx.enter_context(tc.tile_pool(name="otps", bufs=2,
                                              space="PSUM"))

        # ---- persistent constants ----
        wq = []
        for c in range(6):
            t = consts.tile([128, 6 * 384], BF16, tag=f"wq{c}")
            nc.sync.dma_start(t[:], wqkvT[c * 128:(c + 1) * 128, :])
            wq.append(t)
        wp = []
        for c in range(6):
            t = consts.tile([128, C], BF16, tag=f"wp{c}")
            nc.sync.dma_start(t[:], wprojT[c * 128:(c + 1) * 128, :])
            wp.append(t)
        pbias_t = consts.tile([128, 6], F32, tag="pbias")
        nc.sync.dma_start(pbias_t[:], pbias.rearrange("(a p) -> p a", p=128))
        sel12 = consts.tile([12, C], BF16, tag="sel")
        nc.sync.dma_start(sel12[:], sel)
        # tables as [128, NT, D] (partition = token within tile)
        tab_t = []
        for k in range(6):
            t = consts.tile([128, NT, D], BF16, tag=f"tab{k}")
            nc.sync.dma_start(t[:], tabs[k].rearrange("(i p) d -> p i d",
                                                      p=128))
            tab_t.append(t)
        magic_t = consts.tile([128, 24], I32, tag="magic")
        nc.vector.memset(magic_t[:], 0x5F3759DF)

        # q/k feature-major strips: [128, 12, TPAD]; strip s<6 = q pair s,
        # s>=6 = k pair s-6; within strip: rows [0:64]=even head, [64:128]=odd
        qkT = strips.tile([128, 12, TPAD], BF16, tag="qkT", name="qkT")

        # ---- phase A: one 128-token tile ----
        MAGIC = 0x5F3759DF

        def emit_tile(i):
            xg = work.tile([128, 6, 128], BF16, tag="xg", name="xg")
            nc.sync.dma_start(
                xg[:],
                xT.rearrange("(c p) t -> p c t",
                             p=128)[:, :, i * 128:(i + 1) * 128])

            rot = rotp.tile([128, 2, H, D], BF16, tag="rot", name="rot")
            u = stat.tile([128, 24], F32, tag="u")
            rotus = []
            for half in range(2):  # 0 = q, 1 = k
                yc = ypool.tile([128, H, D], BF16, tag="yz", name="yc")
                sq = ypool.tile([128, H, D], BF16, tag="sq", name="sq")
                for jl in range(2):
                    ps = mmps.tile([128, 512], F32, tag="mm")
                    j = half * 2 + jl
                    for c in range(6):
                        nc.tensor.matmul(ps[:, :384], lhsT=xg[:, c, :],
                                         rhs=wq[c][:, j * 384:(j + 1) * 384],
                                         start=(c == 0), stop=(c == 5))
                    # raw copy is the only PSUM reader: fast slot release,
                    # no dependency on the stats chain (ScalarE has only
                    # Exp/Copy/Identity -> single activation table set)
                    nc.scalar.copy(
                        yc[:, jl * 6:(jl + 1) * 6, :],
                        ps[:, :384].rearrange("p (h d) -> p h d", d=D))
                # stats: u = sum(yc^2) = biased variance (weights are /8)
                nc.vector.tensor_mul(sq[:], yc[:], yc[:])
                nc.vector.tensor_reduce(
                    out=u[:, half * 12:(half + 1) * 12], in_=sq[:],
                    axis=mybir.AxisListType.X, op=OP.add)
                # rope on raw values (R(g*(y*s)+b) = s*R(g*y) + R(b));
                # k tables carry the extra x8 so s2 is rsqrt(u+eps) for both
                cct = tab_t[3 * half]
                sst = tab_t[3 * half + 1]
                p2 = ypool.tile([128, H, D], BF16, tag="sq", name="p2")
                nc.gpsimd.tensor_tensor(
                    out=p2[:, :, 0:32], in0=yc[:, :, 32:64],
                    in1=sst[:, i, None, 0:32].broadcast_to([128, H, 32]),
                    op=OP.mult)
                nc.gpsimd.tensor_tensor(
                    out=p2[:, :, 32:64], in0=yc[:, :, 0:32],
                    in1=sst[:, i, None, 32:64].broadcast_to([128, H, 32]),
                    op=OP.mult)
                p1 = ypool.tile([128, H, D], BF16, tag="yz", name="p1")
                nc.vector.tensor_tensor(
                    out=p1[:], in0=yc[:],
                    in1=cct[:, i, None, :].broadcast_to([128, H, D]),
                    op=OP.mult)
                rotu = ypool.tile([128, H, D], BF16, tag="ru", name="rotu")
                nc.vector.tensor_add(rotu[:], p1[:], p2[:])
                rotus.append(rotu)

            # merged rsqrt: s2 = rsqrt(u + eps) via DVE magic + 1 Newton
            vpe = stat.tile([128, 24], F32, tag="vpe")
            srs = stat.tile([128, 24], F32, tag="srs")
            nt1 = stat.tile([128, 24], F32, tag="nt1")
            nc.vector.tensor_scalar(out=vpe[:], in0=u[:], scalar1=float(EPS),
                                    scalar2=None, op0=OP.add)
            nc.vector.tensor_scalar(out=srs[:].bitcast(I32),
                                    in0=vpe[:].bitcast(I32), scalar1=1,
                                    scalar2=None,
                                    op0=OP.logical_shift_right)
            nc.vector.tensor_tensor(out=srs[:].bitcast(I32), in0=magic_t[:],
                                    in1=srs[:].bitcast(I32), op=OP.subtract)
            nc.vector.tensor_mul(nt1[:], srs[:], srs[:])
            nc.vector.tensor_mul(nt1[:], nt1[:], vpe[:])
            nc.vector.tensor_scalar(out=nt1[:], in0=nt1[:], scalar1=-0.5,
                                    scalar2=1.5, op0=OP.mult, op1=OP.add)
            nc.vector.tensor_mul(srs[:], srs[:], nt1[:])

            for half in range(2):
                b2t = tab_t[3 * half + 2]
                rots = ypool.tile([128, H, D], BF16, tag="sq", name="rots")
                nc.vector.scalar_tensor_tensor(
                    out=rots[:], in0=rotus[half][:], scalar=1.0,
                    in1=srs[:, half * 12:(half + 1) * 12,
                            None].broadcast_to([128, H, D]),
                    op0=OP.mult, op1=OP.mult)
                nc.vector.tensor_tensor(
                    out=rot[:, half, :, :], in0=rots[:],
                    in1=b2t[:, i, None, :].broadcast_to([128, H, D]),
                    op=OP.add)

            nc.sync.dma_start_transpose(
                qkT[:, :, i * 128:(i + 1) * 128],
                rot[:].rearrange("p a h d -> p (a h d)"))

        # ---- phase B: one batch = generator of steps ----
        sums_pool = bpool

        def emit_batch(b):
            v65 = []
            for ik, (r0, rows) in enumerate(kblocks):
                g0 = b * N + r0
                xb = work.tile([128, 6, 128], BF16, tag="xb", name="xb")
                nc.sync.dma_start(
                    xb[:, :, :rows],
                    xT.rearrange("(c p) t -> p c t", p=128)[:, :, g0:g0 + rows])
                v = vpool.tile([128, H, D + 1], BF16, tag="v65")
                nc.vector.memset(v[:, :, D:D + 1], 1.0)
                for j in range(2):  # chunks 4,5 = v heads 0-5, 6-11
                    ps = mmps.tile([128, 512], F32, tag="mm")
                    for c in range(6):
                        nc.tensor.matmul(
                            ps[:rows, :384], lhsT=xb[:, c, :rows],
                            rhs=wq[c][:, (4 + j) * 384:(5 + j) * 384],
                            start=(c == 0), stop=(c == 5))
                    nc.scalar.copy(
                        v[:rows, j * 6:(j + 1) * 6, :D],
                        ps[:rows, :384].rearrange("p (h d) -> p h d", d=D))
                v65.append(v)
                yield

            sums_sb = sums_pool.tile([12, N], F32, tag="sums_sb")
            pts_all = [None] * 6
            otsb = [None] * 6
            otn = [None] * 6

            def norm_group():
                rinv = bpool.tile([12, N], F32, tag="rinv")
                rinvb = bpool.tile([12, N], BF16, tag="rinvb")
                nc.vector.reciprocal(rinv[:], sums_sb[:])
                nc.vector.tensor_copy(rinvb[:], rinv[:])
                yield
                for p in range(6):
                    o = otnpool.tile([128, N], BF16, tag="otn")
                    rb = otps.tile([128, 512], F32, tag="ot")
                    nc.tensor.matmul(
                        rb[:, :N],
                        lhsT=sel12[:, p * 128:(p + 1) * 128],
                        rhs=rinvb[:], start=True, stop=True)
                    nc.vector.tensor_mul(o[:], otsb[p][:], rb[:, :N])
                    otn[p] = o
                    if p == 2:
                        yield
                yield

            for p in range(6):
                # scores + exp for pair p
                pts = []
                for (r0, rows) in kblocks:
                    sc = scps.tile([128, 2, 512], F32, tag="sc")
                    kc = b * N + r0
                    for h in range(2):
                        nc.tensor.matmul(
                            sc[:rows, h, :N],
                            lhsT=qkT[h * D:(h + 1) * D, 6 + p, kc:kc + rows],
                            rhs=qkT[h * D:(h + 1) * D, p,
                                    b * N:(b + 1) * N],
                            start=True, stop=True,
                            tile_position=(h * D, 0))
                    pt = ptpool.tile([128, 2, N], BF16, tag="pt")
                    nc.scalar.activation(pt[:rows, :, :], sc[:rows, :, :N],
                                         AF.Exp)
                    pts.append(pt)
                pts_all[p] = pts
                yield

                # AV for pair p (lagged one pair behind scores emission
                # happens naturally via the generator pump order)
                osb = otnpool.tile([128, N], BF16, tag="otsb")
                for h in range(2):
                    hh = 2 * p + h
                    ot = otps.tile([128, 512], F32, tag="ot")
                    for ik, (r0, rows) in enumerate(kblocks):
                        nc.tensor.matmul(
                            ot[:D + 1, :N], lhsT=v65[ik][:rows, hh, :],
                            rhs=pts_all[p][ik][:rows, h, :],
                            start=(ik == 0), stop=(ik == len(kblocks) - 1))
                    stmp = bpool.tile([1, N], F32, tag="stmp", name="stmp")
                    if hh % 2 == 0:
                        nc.scalar.copy(stmp[:], ot[D:D + 1, :N])
                    else:
                        nc.vector.tensor_copy(stmp[:], ot[D:D + 1, :N])
                    nc.sync.dma_start(sums_sb[hh:hh + 1, :], stmp[:])
                    if h == 0:
                        nc.scalar.copy(osb[h * D:(h + 1) * D, :], ot[:D, :N])
                    else:
                        nc.vector.tensor_copy(osb[h * D:(h + 1) * D, :],
                                              ot[:D, :N])
                otsb[p] = osb
                yield

            yield from norm_group()

            # proj + store
            ysb = ysbp.tile([128, 6, N], F32, tag="ysb", name="ysb")
            for co in range(6):
                pp = scps.tile([128, 2, 512], F32, tag="sc", name="pp")
                for cp in range(6):
                    nc.tensor.matmul(
                        pp[:, 0, :N],
                        lhsT=wp[cp][:, co * 128:(co + 1) * 128],
                        rhs=otn[cp][:], start=(cp == 0), stop=(cp == 5))
                nc.scalar.activation(ysb[:, co, :], pp[:, 0, :N], AF.Identity,
                                     bias=pbias_t[:, co:co + 1], scale=1.0)
                if co == 2 or co == 5:
                    yield
            nc.sync.dma_start(
                out.rearrange("(a p) t -> p a t",
                              p=128)[:, :, b * N:(b + 1) * N], ysb[:])

        # ---- interleaved schedule ----
        def last_tile(b):
            return (N * (b + 1) - 1) // 128

        gens = []
        next_b = 0

        def pump(n):
            # advance at most the two oldest generators, alternating, so
            # pool lifetimes never span more than two batches
            nonlocal gens
            k = 0
            idx = 0
            while k < n and gens:
                idx = idx % min(2, len(gens))
                g = gens[idx]
                try:
                    next(g)
                    idx += 1
                except StopIteration:
                    gens.pop(idx)
                k += 1

        for i in range(NT):
            emit_tile(i)
            while next_b < BPC and last_tile(next_b) <= i:
                gens.append(emit_batch(next_b))
                next_b += 1
            pump(6)
        while gens:
            pump(len(gens))

    nc.compile()
    return nc


def _host_tables(rope_tensor, qn_g, qn_b, kn_g, kn_b, P, L):
    """Gain-folded rope tables [4, TPAD, 64]: qCC,qSS,kCC,kSS.

    Deinterleaved lane layout: cols [0:32] = even original lanes,
    [32:64] = odd.  CC = gain*cos; SS = [-g_odd*sin | g_even*sin] so that
    rot = z2*CC + swap_halves(z2)*SS realizes the rotation.
    """
    n_img = N - P - L
    rt = np.asarray(rope_tensor, np.float64)
    cos = rt[:n_img, :, 0]
    sin = rt[:n_img, :, 1]
    c_full = np.ones((N, D // 2))
    s_full = np.zeros((N, D // 2))
    c_full[P:N - L] = cos
    s_full[P:N - L] = sin
    reps = TPAD // N + 2
    c_all = np.tile(c_full, (reps, 1))[:TPAD]
    s_all = np.tile(s_full, (reps, 1))[:TPAD]
    c_all[T:] = 1.0
    s_all[T:] = 0.0

    def mk(g, b, scale):
        g = np.asarray(g, np.float64)
        b = np.asarray(b, np.float64) * scale
        ge, go = g[0::2], g[1::2]
        be, bo = b[0::2], b[1::2]
        CC = np.empty((TPAD, D))
        SS = np.empty((TPAD, D))
        B2 = np.empty((TPAD, D))
        CC[:, 0:32] = ge[None, :] * c_all
        CC[:, 32:64] = go[None, :] * c_all
        SS[:, 0:32] = -go[None, :] * s_all
        SS[:, 32:64] = ge[None, :] * s_all
        B2[:, 0:32] = be[None, :] * c_all - bo[None, :] * s_all
        B2[:, 32:64] = bo[None, :] * c_all + be[None, :] * s_all
        return CC, SS, B2

    qCC, qSS, qB2 = mk(qn_g, qn_b, 1.0 / 8.0)
    kCC, kSS, kB2 = mk(np.asarray(kn_g, np.float64) * 8.0, kn_b, 1.0)
    return np.stack([qCC, qSS, qB2, kCC, kSS, kB2]).astype(BF)


def _host_wqkv(qkv_w):
    """wqkvT [C, 6*384]: chunks 0-1 q, 2-3 k (centered per head, /8,
    deinterleaved lane order), 4-5 v (plain)."""
    wT = np.asarray(qkv_w, np.float64).T  # [C, 3C]
    deint = np.concatenate([np.arange(0, D, 2), np.arange(1, D, 2)])
    outw = np.empty((C, 6 * 384), np.float64)
    for j in range(6):
        cols = wT[:, j * 384:(j + 1) * 384].reshape(C, 6, D)
        if j < 4:  # q, k: center over d per head, scale 1/8, deint
            cols = cols - cols.mean(axis=2, keepdims=True)
            cols = cols[:, :, deint] / 8.0
        outw[:, j * 384:(j + 1) * 384] = cols.reshape(C, 384)
    return outw.astype(BF)


def _host_sel():
    s = np.zeros((12, C), np.float32)
    for k in range(12):
        s[k, k * D:(k + 1) * D] = 1.0
    return s.astype(BF)


def _make_in_maps(x, rope_tensor, qkv_w, proj_w, proj_b, qn_g, qn_b,
                  kn_g, kn_b, P, L):
    tabs = _host_tables(rope_tensor, qn_g, qn_b, kn_g, kn_b, P, L)
    wqkvT = _host_wqkv(qkv_w)
    wprojT = np.ascontiguousarray(
        np.asarray(proj_w, np.float32).T).astype(BF)
    pb = np.ascontiguousarray(np.asarray(proj_b, np.float32))
    sel = _host_sel()
    in_maps = []
    for core in range(NCORES):
        xc = x[core * BPC:(core + 1) * BPC].reshape(T, C)
        xTc = np.zeros((C, TPAD), BF)
        xTc[:, :T] = xc.T.astype(BF)
        in_maps.append({"xT": xTc, "wqkvT": wqkvT, "wprojT": wprojT,
                        "pbias": pb, "tabs": tabs, "sel": sel})
    return in_maps


def kernel(x, rope_tensor, qkv_w, proj_w, proj_b, qn_g, qn_b, kn_g, kn_b,
           num_prefix_tokens, num_latent_tokens, _spmd_kwargs=None):
    P = int(num_prefix_tokens)
    L = int(num_latent_tokens)
    x = np.asarray(x, np.float32)
    assert x.shape == (B, N, C), x.shape

    if "nc" not in _CACHE:
        _CACHE["nc"] = _build_program()
    nc = _CACHE["nc"]

    in_maps = _make_in_maps(x, rope_tensor, qkv_w, proj_w, proj_b,
                            qn_g, qn_b, kn_g, kn_b, P, L)
    res = run_bass_kernel_spmd(nc, in_maps, core_ids=list(range(NCORES)),
                               **(_spmd_kwargs or {}))
    outs = []
    for core in range(NCORES):
        yT = np.asarray(res.results[core]["out"], np.float32)  # [C, T]
        outs.append(yT.T.reshape(BPC, N, C))
    full = np.concatenate(outs, axis=0).astype(np.float32)
    if _spmd_kwargs is not None:
        _CACHE["last_results"] = res
    return full
